# revision 17
# baseline (speedup 1.0000x reference)
"""AttentionBlock (GroupNorm + single-head NxN attention + residual) on 8 TRN2 cores.

Data-parallel: batch dim (B=8) sharded 1 batch-image per NeuronCore. Each core
runs the full block for its image:

  x (C=256, N=4096) -> GroupNorm(8 groups) -> qkv = W_qkv @ xn  ->
  sT = k^T q (scores, keys on partitions), e = exp(sT/16)       ->
  den[n] = sum_m e[m,n] (ones-matmul), attnout_u = v @ e        ->
  proj_u = W_out @ attnout_u; out = proj_u * (1/den) + b_out + x

All heavy matmuls run as float32r (full-rate PE, fp32 storage). The softmax is
computed unnormalized; the 1/den scale commutes through the output projection
and is applied once at the end (per-column broadcast via DMA).
"""

import sys

if "/opt/trn_rl_repo" not in sys.path:
    sys.path.insert(0, "/opt/trn_rl_repo")

import numpy as np

import concourse.bass as bass
import concourse.bacc as bacc
import concourse.tile as tile
import concourse.mybir as mybir
from concourse import bass_utils

# Problem dims (hardcoded per spec)
B, C, HH, WW = 8, 256, 64, 64
N = HH * WW            # 4096
G = 8                  # groupnorm groups
GSZ = C // G           # 32 channels/group
EPS = 1e-5
P = 128                # SBUF partitions
CT = C // P            # 2 channel tiles
NCH = 512              # query-chunk width (free dim per matmul)
NNCH = N // NCH        # 8
MT = N // P            # 32 key tiles
SCALE = 1.0 / np.sqrt(C)
INV_CNT = 1.0 / (GSZ * N)

F32 = mybir.dt.float32
F32R = mybir.dt.float32r


def _emit(tc, d, out_d):
    from contextlib import ExitStack

    nc = tc.nc
    AF = mybir.ActivationFunctionType
    OP = mybir.AluOpType
    AX = mybir.AxisListType.X
    ts, ds = bass.ts, bass.ds

    with ExitStack() as ctx:
        const = ctx.enter_context(tc.tile_pool(name="const", bufs=1))
        big = ctx.enter_context(tc.tile_pool(name="big", bufs=1))
        work = ctx.enter_context(tc.tile_pool(name="work", bufs=3))
        small = ctx.enter_context(tc.tile_pool(name="small", bufs=4))
        outp = ctx.enter_context(tc.tile_pool(name="outp", bufs=3))
        psS = ctx.enter_context(tc.tile_pool(name="psS", bufs=2, space="PSUM"))
        psP = ctx.enter_context(tc.tile_pool(name="psP", bufs=1, space="PSUM"))
        psA = ctx.enter_context(tc.tile_pool(name="psA", bufs=1, space="PSUM"))
        psD = ctx.enter_context(tc.tile_pool(name="psD", bufs=1, space="PSUM"))

        # ---------------- constants / weights to SBUF ----------------
        wq_sb = const.tile([P, CT, C], F32R, name="wq_sb")
        wk_sb = const.tile([P, CT, C], F32R, name="wk_sb")
        wv_sb = const.tile([P, CT, C], F32R, name="wv_sb")
        wo_sb = const.tile([P, CT, C], F32R, name="wo_sb")
        for sb, dr in ((wq_sb, d["wq_t"]), (wk_sb, d["wk_t"]),
                       (wv_sb, d["wv_t"]), (wo_sb, d["wo_t"])):
            for ch in range(CT):
                nc.sync.dma_start(out=sb[:, ch, :], in_=dr[ts(ch, P), :])

        bq_sb = const.tile([P, CT], F32, name="bq_sb")
        bk_sb = const.tile([P, CT], F32, name="bk_sb")
        bo_sb = const.tile([P, CT], F32, name="bo_sb")
        gw_sb = const.tile([P, CT], F32, name="gw_sb")
        gb_sb = const.tile([P, CT], F32, name="gb_sb")
        for sb, dr in ((bq_sb, d["b_q"]), (bk_sb, d["b_k"]), (bo_sb, d["b_o"]),
                       (gw_sb, d["gn_w"]), (gb_sb, d["gn_b"])):
            for t in range(CT):
                nc.sync.dma_start(out=sb[:, t:t + 1], in_=dr[t])

        bv_sb = const.tile([P, C], F32, name="bv_sb")
        nc.sync.dma_start(out=bv_sb, in_=d["b_v"].partition_broadcast(P))

        fm_sb = const.tile([P, CT, G], F32, name="fm_sb")
        bm_sb = const.tile([G, CT, P], F32, name="bm_sb")
        for t in range(CT):
            nc.sync.dma_start(out=fm_sb[:, t, :], in_=d["fmask"][t])
            nc.sync.dma_start(out=bm_sb[:, t, :], in_=d["bmask"][t])

        ones_sb = const.tile([P, 1], F32R, name="ones_sb")
        nc.sync.dma_start(out=ones_sb, in_=d["ones_col"])
        zero_sb = const.tile([P, 1], F32, name="zero_sb")
        nc.vector.memset(zero_sb, 0.0)
        eps_sb = const.tile([G, 1], F32, name="eps_sb")
        nc.vector.memset(eps_sb, EPS)

        # ---------------- load x ----------------
        x_d = d["x"]
        x_sb = big.tile([P, CT, N], F32, name="x_sb")
        for t in range(CT):
            nc.sync.dma_start(out=x_sb[:, t, :], in_=x_d[ts(t, P), :])

        # ---------------- GroupNorm ----------------
        xn_sb = big.tile([P, CT, N], F32R, name="xn_sb")
        stat = small.tile([P, CT, 2], F32, name="stat")
        for t in range(CT):
            nc.vector.reduce_sum(out=stat[:, t, 0:1], in_=x_sb[:, t, :], axis=AX)
            # x^2 into xn (scratch; overwritten below), row-sum into stat col 1
            nc.scalar.activation(out=xn_sb[:, t, :], in_=x_sb[:, t, :],
                                 func=AF.Square, bias=zero_sb,
                                 accum_out=stat[:, t, 1:2])

        gps = psS.tile([G, 2], F32, tag="s", name="gps")
        for t in range(CT):
            nc.tensor.matmul(gps, lhsT=fm_sb[:, t, :], rhs=stat[:, t, :],
                             start=(t == 0), stop=(t == CT - 1))
        grp = small.tile([G, 2], F32, name="grp")    # [mean, rstd]
        gtmp = small.tile([G, 3], F32, name="gtmp")
        nc.vector.tensor_scalar_mul(out=grp[:, 0:1], in0=gps[:, 0:1], scalar1=INV_CNT)
        nc.vector.tensor_scalar_mul(out=gtmp[:, 0:1], in0=gps[:, 1:2], scalar1=INV_CNT)
        nc.vector.tensor_mul(out=gtmp[:, 1:2], in0=grp[:, 0:1], in1=grp[:, 0:1])
        nc.vector.tensor_sub(out=gtmp[:, 2:3], in0=gtmp[:, 0:1], in1=gtmp[:, 1:2])
        nc.scalar.activation(out=gtmp[:, 2:3], in_=gtmp[:, 2:3], func=AF.Sqrt,
                             bias=eps_sb)
        nc.vector.reciprocal(out=grp[:, 1:2], in_=gtmp[:, 2:3])

        ab = small.tile([P, CT, 2], F32, name="ab")  # per-channel scale a, bias b
        for t in range(CT):
            cps = psS.tile([P, 2], F32, tag="s", name="cps")
            nc.tensor.matmul(cps, lhsT=bm_sb[:, t, :], rhs=grp, start=True, stop=True)
            nc.vector.tensor_mul(out=ab[:, t, 0:1], in0=cps[:, 1:2], in1=gw_sb[:, t:t + 1])
            nc.vector.tensor_mul(out=ab[:, t, 1:2], in0=cps[:, 0:1], in1=ab[:, t, 0:1])
            nc.vector.tensor_sub(out=ab[:, t, 1:2], in0=gb_sb[:, t:t + 1], in1=ab[:, t, 1:2])
            nc.vector.tensor_scalar(out=xn_sb[:, t, :], in0=x_sb[:, t, :],
                                    scalar1=ab[:, t, 0:1], scalar2=ab[:, t, 1:2],
                                    op0=OP.mult, op1=OP.add)

        # ---------------- QKV projections ----------------
        q_sb = big.tile([P, CT, N], F32R, name="q_sb")   # (c_half, n)
        k_sb = big.tile([P, CT, N], F32R, name="k_sb")
        vT_sb = big.tile([P, MT, C], F32R, name="vT_sb")  # (n, c), n on partitions

        for (w_sb, b_sb, o_sb) in ((wq_sb, bq_sb, q_sb), (wk_sb, bk_sb, k_sb)):
            for tq in range(CT):
                for nch in range(NNCH):
                    ps = psS.tile([P, NCH], F32, tag="s", name="psqk")
                    for ch in range(CT):
                        nc.tensor.matmul(
                            ps,
                            lhsT=w_sb[:, ch, ts(tq, P)],
                            rhs=xn_sb[:, ch, ds(nch * NCH, NCH)],
                            start=(ch == 0), stop=(ch == CT - 1))
                    nc.vector.tensor_scalar_add(out=o_sb[:, tq, ds(nch * NCH, NCH)],
                                                in0=ps, scalar1=b_sb[:, tq:tq + 1])

        for mt in range(MT):
            ps = psS.tile([P, C], F32, tag="s", name="psv")
            for ch in range(CT):
                nc.tensor.matmul(ps,
                                 lhsT=xn_sb[:, ch, ts(mt, P)],
                                 rhs=wv_sb[:, ch, :],
                                 start=(ch == 0), stop=(ch == CT - 1))
            nc.vector.tensor_add(out=vT_sb[:, mt, :], in0=ps, in1=bv_sb)

        # ---------------- attention + output projection ----------------
        PAIR = 2  # score tiles per exp instruction
        for nch in range(NNCH):
            nsl = ds(nch * NCH, NCH)
            attn = psA.tile([P, CT, NCH], F32, tag="attn", name="attn")
            den = psD.tile([1, NCH], F32, tag="den", name="den")
            for mt0 in range(0, MT, PAIR):
                s = psS.tile([P, PAIR, NCH], F32, tag="s", name="s")
                for j in range(PAIR):
                    for ch in range(CT):
                        nc.tensor.matmul(s[:, j, :],
                                         lhsT=k_sb[:, ch, ts(mt0 + j, P)],
                                         rhs=q_sb[:, ch, nsl],
                                         start=(ch == 0), stop=(ch == CT - 1))
                e = work.tile([P, PAIR, NCH], F32R, tag="e", name="e")
                nc.scalar.activation(out=e, in_=s, func=AF.Exp, bias=zero_sb,
                                     scale=SCALE)
                for j in range(PAIR):
                    mt = mt0 + j
                    for ch in range(CT):
                        nc.tensor.matmul(attn[:, ch, :],
                                         lhsT=vT_sb[:, mt, ts(ch, P)],
                                         rhs=e[:, j, :],
                                         start=(mt == 0), stop=(mt == MT - 1))
                    nc.tensor.matmul(den, lhsT=ones_sb, rhs=e[:, j, :],
                                     start=(mt == 0), stop=(mt == MT - 1))

            den_sb = small.tile([1, NCH], F32, tag="den_sb", name="den_sb", bufs=2)
            nc.vector.tensor_copy(out=den_sb, in_=den)
            rden = small.tile([1, NCH], F32, tag="rden", name="rden", bufs=2)
            nc.vector.reciprocal(out=rden, in_=den_sb)
            rdenb = outp.tile([P, NCH], F32, tag="rdenb", name="rdenb", bufs=2)
            nc.gpsimd.partition_broadcast(rdenb, rden)

            atts = []
            for ch in range(CT):
                att = outp.tile([P, NCH], F32R, tag="att", name=f"att{ch}", bufs=3)
                nc.vector.tensor_copy(out=att, in_=attn[:, ch, :])
                atts.append(att)

            for co in range(CT):
                pj = psP.tile([P, NCH], F32, tag="proj", name="pj")
                for ch in range(CT):
                    nc.tensor.matmul(pj,
                                     lhsT=wo_sb[:, ch, ts(co, P)],
                                     rhs=atts[ch],
                                     start=(ch == 0), stop=(ch == CT - 1))
                f = outp.tile([P, NCH], F32, tag="fout", name="f", bufs=2)
                nc.vector.tensor_tensor(out=f, in0=pj, in1=rdenb, op=OP.mult)
                nc.vector.scalar_tensor_tensor(out=f, in0=f, scalar=bo_sb[:, co:co + 1],
                                               in1=x_sb[:, co, nsl],
                                               op0=OP.add, op1=OP.add)
                nc.sync.dma_start(out=out_d[ts(co, P), nsl], in_=f)


def build_program():
    nc = bacc.Bacc("TRN2", target_bir_lowering=False, debug=False, num_devices=B)
    d = {}

    def din(name, shape, dt_=F32):
        d[name] = nc.dram_tensor(name, list(shape), dt_, kind="ExternalInput").ap()

    din("x", (C, N))
    din("wq_t", (C, C), F32R)
    din("wk_t", (C, C), F32R)
    din("wv_t", (C, C), F32R)
    din("wo_t", (C, C), F32R)
    din("b_q", (CT, P, 1))
    din("b_k", (CT, P, 1))
    din("b_v", (C,))
    din("b_o", (CT, P, 1))
    din("gn_w", (CT, P, 1))
    din("gn_b", (CT, P, 1))
    din("fmask", (CT, P, G))
    din("bmask", (CT, G, P))
    din("ones_col", (P, 1), F32R)
    out_d = nc.dram_tensor("out", [C, N], F32, kind="ExternalOutput").ap()

    with tile.TileContext(nc) as tc:
        _emit(tc, d, out_d)
    nc.compile()
    return nc


_PROG = None


def _get_program():
    global _PROG
    if _PROG is None:
        _PROG = build_program()
    return _PROG


def make_in_maps(inputs):
    x = np.ascontiguousarray(np.asarray(inputs["x"], dtype=np.float32))
    w_qkv = np.asarray(inputs["w_qkv"], dtype=np.float32)
    b_qkv = np.asarray(inputs["b_qkv"], dtype=np.float32)
    w_out = np.asarray(inputs["w_out"], dtype=np.float32)
    b_out = np.asarray(inputs["b_out"], dtype=np.float32)
    gn_scale = np.asarray(inputs["gn_scale"], dtype=np.float32)
    gn_bias = np.asarray(inputs["gn_bias"], dtype=np.float32)

    fmask = np.zeros((CT, P, G), dtype=np.float32)
    for t in range(CT):
        for p in range(P):
            fmask[t, p, (t * P + p) // GSZ] = 1.0
    bmask = np.ascontiguousarray(fmask.transpose(0, 2, 1))

    common = {
        "wq_t": np.ascontiguousarray(w_qkv[0:C].T),
        "wk_t": np.ascontiguousarray(w_qkv[C:2 * C].T),
        "wv_t": np.ascontiguousarray(w_qkv[2 * C:3 * C].T),
        "wo_t": np.ascontiguousarray(w_out.T),
        "b_q": np.ascontiguousarray(b_qkv[0:C].reshape(CT, P, 1)),
        "b_k": np.ascontiguousarray(b_qkv[C:2 * C].reshape(CT, P, 1)),
        "b_v": np.ascontiguousarray(b_qkv[2 * C:3 * C]),
        "b_o": np.ascontiguousarray(b_out.reshape(CT, P, 1)),
        "gn_w": np.ascontiguousarray(gn_scale.reshape(CT, P, 1)),
        "gn_b": np.ascontiguousarray(gn_bias.reshape(CT, P, 1)),
        "fmask": fmask,
        "bmask": bmask,
        "ones_col": np.ones((P, 1), dtype=np.float32),
    }
    return [dict(common, x=np.ascontiguousarray(x[b].reshape(C, N)))
            for b in range(B)]


def run(inputs, trace=False):
    nc = _get_program()
    in_maps = make_in_maps(inputs)
    res = bass_utils.run_bass_kernel_spmd(nc, in_maps, core_ids=list(range(B)),
                                          trace=trace)
    out = np.stack([res.results[b]["out"] for b in range(B)])
    return out.reshape(B, C, HH, WW), res


def kernel(**inputs):
    out, _ = run(inputs, trace=False)
    return out


# revision 18
# speedup vs baseline: 1.1537x; 1.1537x over previous
"""AttentionBlock (GroupNorm + single-head NxN attention + residual) on 8 TRN2 cores.

Data-parallel: batch dim (B=8) sharded 1 batch-image per NeuronCore. Each core
runs the full block for its image:

  x (C=256, N=4096) -> GroupNorm(8 groups) -> qkv = W_qkv @ xn  ->
  sT = k^T q (scores, keys on partitions), e = exp(sT/16)       ->
  den[n] = sum_m e[m,n] (ones-matmul), attnout_u = v @ e        ->
  proj_u = W_out @ attnout_u; out = proj_u * (1/den) + b_out + x

All heavy matmuls run as float32r (full-rate PE, fp32 storage). The softmax is
computed unnormalized; the 1/den scale commutes through the output projection
and is applied once at the end (per-column broadcast via DMA).
"""

import sys

if "/opt/trn_rl_repo" not in sys.path:
    sys.path.insert(0, "/opt/trn_rl_repo")

import numpy as np

import concourse.bass as bass
import concourse.bacc as bacc
import concourse.tile as tile
import concourse.mybir as mybir
from concourse import bass_utils

# Problem dims (hardcoded per spec)
B, C, HH, WW = 8, 256, 64, 64
N = HH * WW            # 4096
G = 8                  # groupnorm groups
GSZ = C // G           # 32 channels/group
EPS = 1e-5
P = 128                # SBUF partitions
CT = C // P            # 2 channel tiles
NCH = 512              # query-chunk width (free dim per matmul)
NNCH = N // NCH        # 8
MT = N // P            # 32 key tiles
SCALE = 1.0 / np.sqrt(C)
INV_CNT = 1.0 / (GSZ * N)

F32 = mybir.dt.float32
F32R = mybir.dt.float32r


def _emit(tc, d, out_d):
    from contextlib import ExitStack

    nc = tc.nc
    AF = mybir.ActivationFunctionType
    OP = mybir.AluOpType
    AX = mybir.AxisListType.X
    ts, ds = bass.ts, bass.ds

    with ExitStack() as ctx:
        const = ctx.enter_context(tc.tile_pool(name="const", bufs=1))
        big = ctx.enter_context(tc.tile_pool(name="big", bufs=1))
        work = ctx.enter_context(tc.tile_pool(name="work", bufs=3))
        small = ctx.enter_context(tc.tile_pool(name="small", bufs=4))
        outp = ctx.enter_context(tc.tile_pool(name="outp", bufs=3))
        psS = ctx.enter_context(tc.tile_pool(name="psS", bufs=2, space="PSUM"))
        psP = ctx.enter_context(tc.tile_pool(name="psP", bufs=2, space="PSUM"))
        psA = ctx.enter_context(tc.tile_pool(name="psA", bufs=1, space="PSUM"))
        psD = ctx.enter_context(tc.tile_pool(name="psD", bufs=1, space="PSUM"))

        # ---------------- constants / weights to SBUF ----------------
        wq_sb = const.tile([P, CT, C], F32R, name="wq_sb")
        wk_sb = const.tile([P, CT, C], F32R, name="wk_sb")
        wv_sb = const.tile([P, CT, C], F32R, name="wv_sb")
        wo_sb = const.tile([P, CT, C], F32R, name="wo_sb")
        for sb, dr in ((wq_sb, d["wq_t"]), (wk_sb, d["wk_t"]),
                       (wv_sb, d["wv_t"]), (wo_sb, d["wo_t"])):
            for ch in range(CT):
                nc.sync.dma_start(out=sb[:, ch, :], in_=dr[ts(ch, P), :])

        bq_sb = const.tile([P, CT], F32, name="bq_sb")
        bk_sb = const.tile([P, CT], F32, name="bk_sb")
        bo_sb = const.tile([P, CT], F32, name="bo_sb")
        gw_sb = const.tile([P, CT], F32, name="gw_sb")
        gb_sb = const.tile([P, CT], F32, name="gb_sb")
        for sb, dr in ((bq_sb, d["b_q"]), (bk_sb, d["b_k"]), (bo_sb, d["b_o"]),
                       (gw_sb, d["gn_w"]), (gb_sb, d["gn_b"])):
            for t in range(CT):
                nc.sync.dma_start(out=sb[:, t:t + 1], in_=dr[t])

        bv_sb = const.tile([P, C], F32, name="bv_sb")
        nc.sync.dma_start(out=bv_sb, in_=d["b_v"].partition_broadcast(P))

        fm_sb = const.tile([P, CT, G], F32, name="fm_sb")
        bm_sb = const.tile([G, CT, P], F32, name="bm_sb")
        for t in range(CT):
            nc.sync.dma_start(out=fm_sb[:, t, :], in_=d["fmask"][t])
            nc.sync.dma_start(out=bm_sb[:, t, :], in_=d["bmask"][t])

        ones_sb = const.tile([P, 1], F32R, name="ones_sb")
        nc.sync.dma_start(out=ones_sb, in_=d["ones_col"])
        zero_sb = const.tile([P, 1], F32, name="zero_sb")
        nc.vector.memset(zero_sb, 0.0)
        eps_sb = const.tile([G, 1], F32, name="eps_sb")
        nc.vector.memset(eps_sb, EPS)

        # ---------------- load x ----------------
        x_d = d["x"]
        x_sb = big.tile([P, CT, N], F32, name="x_sb")
        for t in range(CT):
            nc.sync.dma_start(out=x_sb[:, t, :], in_=x_d[ts(t, P), :])

        # ---------------- GroupNorm ----------------
        xn_sb = big.tile([P, CT, N], F32R, name="xn_sb")
        stat = small.tile([P, CT, 2], F32, name="stat")
        for t in range(CT):
            nc.vector.reduce_sum(out=stat[:, t, 0:1], in_=x_sb[:, t, :], axis=AX)
            # x^2 into xn (scratch; overwritten below), row-sum into stat col 1
            nc.scalar.activation(out=xn_sb[:, t, :], in_=x_sb[:, t, :],
                                 func=AF.Square, bias=zero_sb,
                                 accum_out=stat[:, t, 1:2])

        gps = psS.tile([G, 2], F32, tag="s", name="gps")
        for t in range(CT):
            nc.tensor.matmul(gps, lhsT=fm_sb[:, t, :], rhs=stat[:, t, :],
                             start=(t == 0), stop=(t == CT - 1))
        grp = small.tile([G, 2], F32, name="grp")    # [mean, rstd]
        gtmp = small.tile([G, 3], F32, name="gtmp")
        nc.vector.tensor_scalar_mul(out=grp[:, 0:1], in0=gps[:, 0:1], scalar1=INV_CNT)
        nc.vector.tensor_scalar_mul(out=gtmp[:, 0:1], in0=gps[:, 1:2], scalar1=INV_CNT)
        nc.vector.tensor_mul(out=gtmp[:, 1:2], in0=grp[:, 0:1], in1=grp[:, 0:1])
        nc.vector.tensor_sub(out=gtmp[:, 2:3], in0=gtmp[:, 0:1], in1=gtmp[:, 1:2])
        nc.scalar.activation(out=gtmp[:, 2:3], in_=gtmp[:, 2:3], func=AF.Sqrt,
                             bias=eps_sb)
        nc.vector.reciprocal(out=grp[:, 1:2], in_=gtmp[:, 2:3])

        ab = small.tile([P, CT, 2], F32, name="ab")  # per-channel scale a, bias b
        for t in range(CT):
            cps = psS.tile([P, 2], F32, tag="s", name="cps")
            nc.tensor.matmul(cps, lhsT=bm_sb[:, t, :], rhs=grp, start=True, stop=True)
            nc.vector.tensor_mul(out=ab[:, t, 0:1], in0=cps[:, 1:2], in1=gw_sb[:, t:t + 1])
            nc.vector.tensor_mul(out=ab[:, t, 1:2], in0=cps[:, 0:1], in1=ab[:, t, 0:1])
            nc.vector.tensor_sub(out=ab[:, t, 1:2], in0=gb_sb[:, t:t + 1], in1=ab[:, t, 1:2])
            nc.vector.tensor_scalar(out=xn_sb[:, t, :], in0=x_sb[:, t, :],
                                    scalar1=ab[:, t, 0:1], scalar2=ab[:, t, 1:2],
                                    op0=OP.mult, op1=OP.add)

        # ---------------- QKV projections ----------------
        q_sb = big.tile([P, CT, N], F32R, name="q_sb")   # (c_half, n)
        k_sb = big.tile([P, CT, N], F32R, name="k_sb")
        vT_sb = big.tile([P, MT, C], F32R, name="vT_sb")  # (n, c), n on partitions

        for (w_sb, b_sb, o_sb) in ((wq_sb, bq_sb, q_sb), (wk_sb, bk_sb, k_sb)):
            for tq in range(CT):
                for nch in range(NNCH):
                    ps = psS.tile([P, NCH], F32, tag="s", name="psqk")
                    for ch in range(CT):
                        nc.tensor.matmul(
                            ps,
                            lhsT=w_sb[:, ch, ts(tq, P)],
                            rhs=xn_sb[:, ch, ds(nch * NCH, NCH)],
                            start=(ch == 0), stop=(ch == CT - 1))
                    nc.vector.tensor_scalar_add(out=o_sb[:, tq, ds(nch * NCH, NCH)],
                                                in0=ps, scalar1=b_sb[:, tq:tq + 1])

        for mt in range(MT):
            ps = psS.tile([P, C], F32, tag="s", name="psv")
            for ch in range(CT):
                nc.tensor.matmul(ps,
                                 lhsT=xn_sb[:, ch, ts(mt, P)],
                                 rhs=wv_sb[:, ch, :],
                                 start=(ch == 0), stop=(ch == CT - 1))
            nc.vector.tensor_add(out=vT_sb[:, mt, :], in0=ps, in1=bv_sb)

        # ---------------- attention + output projection ----------------
        PAIR = 1  # score tiles per exp instruction
        for nch in range(NNCH):
            nsl = ds(nch * NCH, NCH)
            attn = psA.tile([P, CT, NCH], F32, tag="attn", name="attn")
            den = psD.tile([1, NCH], F32, tag="den", name="den")
            for mt0 in range(0, MT, PAIR):
                s = psS.tile([P, PAIR, NCH], F32, tag="s", name="s")
                for j in range(PAIR):
                    for ch in range(CT):
                        nc.tensor.matmul(s[:, j, :],
                                         lhsT=k_sb[:, ch, ts(mt0 + j, P)],
                                         rhs=q_sb[:, ch, nsl],
                                         start=(ch == 0), stop=(ch == CT - 1))
                e = work.tile([P, PAIR, NCH], F32R, tag="e", name="e")
                nc.scalar.activation(out=e, in_=s, func=AF.Exp, bias=zero_sb,
                                     scale=SCALE)
                for j in range(PAIR):
                    mt = mt0 + j
                    for ch in range(CT):
                        nc.tensor.matmul(attn[:, ch, :],
                                         lhsT=vT_sb[:, mt, ts(ch, P)],
                                         rhs=e[:, j, :],
                                         start=(mt == 0), stop=(mt == MT - 1))
                    nc.tensor.matmul(den, lhsT=ones_sb, rhs=e[:, j, :],
                                     start=(mt == 0), stop=(mt == MT - 1))

            den_sb = small.tile([1, NCH], F32, tag="den_sb", name="den_sb", bufs=2)
            nc.vector.tensor_copy(out=den_sb, in_=den)
            rden = small.tile([1, NCH], F32, tag="rden", name="rden", bufs=2)
            nc.vector.reciprocal(out=rden, in_=den_sb)
            rdenb = outp.tile([P, NCH], F32, tag="rdenb", name="rdenb", bufs=2)
            nc.gpsimd.partition_broadcast(rdenb, rden)

            atts = []
            for ch in range(CT):
                att = outp.tile([P, NCH], F32R, tag="att", name=f"att{ch}", bufs=3)
                nc.vector.tensor_copy(out=att, in_=attn[:, ch, :])
                atts.append(att)

            for co in range(CT):
                pj = psP.tile([P, NCH], F32, tag="proj", name="pj")
                for ch in range(CT):
                    nc.tensor.matmul(pj,
                                     lhsT=wo_sb[:, ch, ts(co, P)],
                                     rhs=atts[ch],
                                     start=(ch == 0), stop=(ch == CT - 1))
                f = outp.tile([P, NCH], F32, tag="fout", name="f", bufs=2)
                nc.vector.tensor_tensor(out=f, in0=pj, in1=rdenb, op=OP.mult)
                nc.vector.scalar_tensor_tensor(out=f, in0=f, scalar=bo_sb[:, co:co + 1],
                                               in1=x_sb[:, co, nsl],
                                               op0=OP.add, op1=OP.add)
                nc.sync.dma_start(out=out_d[ts(co, P), nsl], in_=f)


def build_program():
    nc = bacc.Bacc("TRN2", target_bir_lowering=False, debug=False, num_devices=B)
    d = {}

    def din(name, shape, dt_=F32):
        d[name] = nc.dram_tensor(name, list(shape), dt_, kind="ExternalInput").ap()

    din("x", (C, N))
    din("wq_t", (C, C), F32R)
    din("wk_t", (C, C), F32R)
    din("wv_t", (C, C), F32R)
    din("wo_t", (C, C), F32R)
    din("b_q", (CT, P, 1))
    din("b_k", (CT, P, 1))
    din("b_v", (C,))
    din("b_o", (CT, P, 1))
    din("gn_w", (CT, P, 1))
    din("gn_b", (CT, P, 1))
    din("fmask", (CT, P, G))
    din("bmask", (CT, G, P))
    din("ones_col", (P, 1), F32R)
    out_d = nc.dram_tensor("out", [C, N], F32, kind="ExternalOutput").ap()

    with tile.TileContext(nc) as tc:
        _emit(tc, d, out_d)
    nc.compile()
    return nc


_PROG = None


def _get_program():
    global _PROG
    if _PROG is None:
        _PROG = build_program()
    return _PROG


def make_in_maps(inputs):
    x = np.ascontiguousarray(np.asarray(inputs["x"], dtype=np.float32))
    w_qkv = np.asarray(inputs["w_qkv"], dtype=np.float32)
    b_qkv = np.asarray(inputs["b_qkv"], dtype=np.float32)
    w_out = np.asarray(inputs["w_out"], dtype=np.float32)
    b_out = np.asarray(inputs["b_out"], dtype=np.float32)
    gn_scale = np.asarray(inputs["gn_scale"], dtype=np.float32)
    gn_bias = np.asarray(inputs["gn_bias"], dtype=np.float32)

    fmask = np.zeros((CT, P, G), dtype=np.float32)
    for t in range(CT):
        for p in range(P):
            fmask[t, p, (t * P + p) // GSZ] = 1.0
    bmask = np.ascontiguousarray(fmask.transpose(0, 2, 1))

    common = {
        "wq_t": np.ascontiguousarray(w_qkv[0:C].T),
        "wk_t": np.ascontiguousarray(w_qkv[C:2 * C].T),
        "wv_t": np.ascontiguousarray(w_qkv[2 * C:3 * C].T),
        "wo_t": np.ascontiguousarray(w_out.T),
        "b_q": np.ascontiguousarray(b_qkv[0:C].reshape(CT, P, 1)),
        "b_k": np.ascontiguousarray(b_qkv[C:2 * C].reshape(CT, P, 1)),
        "b_v": np.ascontiguousarray(b_qkv[2 * C:3 * C]),
        "b_o": np.ascontiguousarray(b_out.reshape(CT, P, 1)),
        "gn_w": np.ascontiguousarray(gn_scale.reshape(CT, P, 1)),
        "gn_b": np.ascontiguousarray(gn_bias.reshape(CT, P, 1)),
        "fmask": fmask,
        "bmask": bmask,
        "ones_col": np.ones((P, 1), dtype=np.float32),
    }
    return [dict(common, x=np.ascontiguousarray(x[b].reshape(C, N)))
            for b in range(B)]


def run(inputs, trace=False):
    nc = _get_program()
    in_maps = make_in_maps(inputs)
    res = bass_utils.run_bass_kernel_spmd(nc, in_maps, core_ids=list(range(B)),
                                          trace=trace)
    out = np.stack([res.results[b]["out"] for b in range(B)])
    return out.reshape(B, C, HH, WW), res


def kernel(**inputs):
    out, _ = run(inputs, trace=False)
    return out


# revision 19
# speedup vs baseline: 1.3594x; 1.1783x over previous
"""AttentionBlock (GroupNorm + single-head NxN attention + residual) on 8 TRN2 cores.

Data-parallel: batch dim (B=8) sharded 1 batch-image per NeuronCore. Each core
runs the full block for its image:

  x (C=256, N=4096) -> GroupNorm(8 groups) -> qkv = W_qkv @ xn  ->
  sT = k^T q (scores, keys on partitions), e = exp(sT/16)       ->
  den[n] = sum_m e[m,n] (ones-matmul), attnout_u = v @ e        ->
  proj_u = W_out @ attnout_u; out = proj_u * (1/den) + b_out + x

All heavy matmuls run as float32r (full-rate PE, fp32 storage). The softmax is
computed unnormalized; the 1/den scale commutes through the output projection
and is applied once at the end (per-column broadcast via DMA).
"""

import sys

if "/opt/trn_rl_repo" not in sys.path:
    sys.path.insert(0, "/opt/trn_rl_repo")

import numpy as np

import concourse.bass as bass
import concourse.bacc as bacc
import concourse.tile as tile
import concourse.mybir as mybir
from concourse import bass_utils

# Problem dims (hardcoded per spec)
B, C, HH, WW = 8, 256, 64, 64
N = HH * WW            # 4096
G = 8                  # groupnorm groups
GSZ = C // G           # 32 channels/group
EPS = 1e-5
P = 128                # SBUF partitions
CT = C // P            # 2 channel tiles
NCH = 512              # query-chunk width (free dim per matmul)
NNCH = N // NCH        # 8
MT = N // P            # 32 key tiles
SCALE = 1.0 / np.sqrt(C)
INV_CNT = 1.0 / (GSZ * N)

F32 = mybir.dt.float32
F32R = mybir.dt.float32r


def _emit(tc, d, out_d):
    from contextlib import ExitStack

    nc = tc.nc
    AF = mybir.ActivationFunctionType
    OP = mybir.AluOpType
    AX = mybir.AxisListType.X
    ts, ds = bass.ts, bass.ds

    with ExitStack() as ctx:
        const = ctx.enter_context(tc.tile_pool(name="const", bufs=1))
        big = ctx.enter_context(tc.tile_pool(name="big", bufs=1))
        work = ctx.enter_context(tc.tile_pool(name="work", bufs=3))
        small = ctx.enter_context(tc.tile_pool(name="small", bufs=4))
        outp = ctx.enter_context(tc.tile_pool(name="outp", bufs=3))
        psS = ctx.enter_context(tc.tile_pool(name="psS", bufs=3, space="PSUM"))
        psP = ctx.enter_context(tc.tile_pool(name="psP", bufs=2, space="PSUM"))
        psA = ctx.enter_context(tc.tile_pool(name="psA", bufs=1, space="PSUM"))
        psD = ctx.enter_context(tc.tile_pool(name="psD", bufs=1, space="PSUM"))

        # ---------------- constants / weights to SBUF ----------------
        wq_sb = const.tile([P, CT, C], F32R, name="wq_sb")
        wk_sb = const.tile([P, CT, C], F32R, name="wk_sb")
        wv_sb = const.tile([P, CT, C], F32R, name="wv_sb")
        wo_sb = const.tile([P, CT, C], F32R, name="wo_sb")
        for sb, dr in ((wq_sb, d["wq_t"]), (wk_sb, d["wk_t"]),
                       (wv_sb, d["wv_t"]), (wo_sb, d["wo_t"])):
            for ch in range(CT):
                nc.sync.dma_start(out=sb[:, ch, :], in_=dr[ts(ch, P), :])

        bq_sb = const.tile([P, CT], F32, name="bq_sb")
        bk_sb = const.tile([P, CT], F32, name="bk_sb")
        bo_sb = const.tile([P, CT], F32, name="bo_sb")
        gw_sb = const.tile([P, CT], F32, name="gw_sb")
        gb_sb = const.tile([P, CT], F32, name="gb_sb")
        for sb, dr in ((bq_sb, d["b_q"]), (bk_sb, d["b_k"]), (bo_sb, d["b_o"]),
                       (gw_sb, d["gn_w"]), (gb_sb, d["gn_b"])):
            for t in range(CT):
                nc.sync.dma_start(out=sb[:, t:t + 1], in_=dr[t])

        bv_sb = const.tile([P, C], F32, name="bv_sb")
        nc.sync.dma_start(out=bv_sb, in_=d["b_v"].partition_broadcast(P))

        fm_sb = const.tile([P, CT, G], F32, name="fm_sb")
        bm_sb = const.tile([G, CT, P], F32, name="bm_sb")
        for t in range(CT):
            nc.sync.dma_start(out=fm_sb[:, t, :], in_=d["fmask"][t])
            nc.sync.dma_start(out=bm_sb[:, t, :], in_=d["bmask"][t])

        ones_sb = const.tile([P, 1], F32R, name="ones_sb")
        nc.sync.dma_start(out=ones_sb, in_=d["ones_col"])
        zero_sb = const.tile([P, 1], F32, name="zero_sb")
        nc.vector.memset(zero_sb, 0.0)
        eps_sb = const.tile([G, 1], F32, name="eps_sb")
        nc.vector.memset(eps_sb, EPS)

        # ---------------- load x ----------------
        x_d = d["x"]
        x_sb = big.tile([P, CT, N], F32, name="x_sb")
        for t in range(CT):
            nc.sync.dma_start(out=x_sb[:, t, :], in_=x_d[ts(t, P), :])

        # ---------------- GroupNorm ----------------
        xn_sb = big.tile([P, CT, N], F32R, name="xn_sb")
        stat = small.tile([P, CT, 2], F32, name="stat")
        for t in range(CT):
            nc.vector.reduce_sum(out=stat[:, t, 0:1], in_=x_sb[:, t, :], axis=AX)
            # x^2 into xn (scratch; overwritten below), row-sum into stat col 1
            nc.scalar.activation(out=xn_sb[:, t, :], in_=x_sb[:, t, :],
                                 func=AF.Square, bias=zero_sb,
                                 accum_out=stat[:, t, 1:2])

        gps = psS.tile([G, 2], F32, tag="s", name="gps")
        for t in range(CT):
            nc.tensor.matmul(gps, lhsT=fm_sb[:, t, :], rhs=stat[:, t, :],
                             start=(t == 0), stop=(t == CT - 1))
        grp = small.tile([G, 2], F32, name="grp")    # [mean, rstd]
        gtmp = small.tile([G, 3], F32, name="gtmp")
        nc.vector.tensor_scalar_mul(out=grp[:, 0:1], in0=gps[:, 0:1], scalar1=INV_CNT)
        nc.vector.tensor_scalar_mul(out=gtmp[:, 0:1], in0=gps[:, 1:2], scalar1=INV_CNT)
        nc.vector.tensor_mul(out=gtmp[:, 1:2], in0=grp[:, 0:1], in1=grp[:, 0:1])
        nc.vector.tensor_sub(out=gtmp[:, 2:3], in0=gtmp[:, 0:1], in1=gtmp[:, 1:2])
        nc.scalar.activation(out=gtmp[:, 2:3], in_=gtmp[:, 2:3], func=AF.Sqrt,
                             bias=eps_sb)
        nc.vector.reciprocal(out=grp[:, 1:2], in_=gtmp[:, 2:3])

        ab = small.tile([P, CT, 2], F32, name="ab")  # per-channel scale a, bias b
        for t in range(CT):
            cps = psS.tile([P, 2], F32, tag="s", name="cps")
            nc.tensor.matmul(cps, lhsT=bm_sb[:, t, :], rhs=grp, start=True, stop=True)
            nc.vector.tensor_mul(out=ab[:, t, 0:1], in0=cps[:, 1:2], in1=gw_sb[:, t:t + 1])
            nc.vector.tensor_mul(out=ab[:, t, 1:2], in0=cps[:, 0:1], in1=ab[:, t, 0:1])
            nc.vector.tensor_sub(out=ab[:, t, 1:2], in0=gb_sb[:, t:t + 1], in1=ab[:, t, 1:2])
            nc.vector.tensor_scalar(out=xn_sb[:, t, :], in0=x_sb[:, t, :],
                                    scalar1=ab[:, t, 0:1], scalar2=ab[:, t, 1:2],
                                    op0=OP.mult, op1=OP.add)

        # ---------------- QKV projections ----------------
        q_sb = big.tile([P, CT, N], F32R, name="q_sb")   # (c_half, n)
        k_sb = big.tile([P, CT, N], F32R, name="k_sb")
        vT_sb = big.tile([P, MT, C], F32R, name="vT_sb")  # (n, c), n on partitions

        for (w_sb, b_sb, o_sb) in ((wq_sb, bq_sb, q_sb), (wk_sb, bk_sb, k_sb)):
            for tq in range(CT):
                for nch in range(NNCH):
                    ps = psS.tile([P, NCH], F32, tag="s", name="psqk")
                    for ch in range(CT):
                        nc.tensor.matmul(
                            ps,
                            lhsT=w_sb[:, ch, ts(tq, P)],
                            rhs=xn_sb[:, ch, ds(nch * NCH, NCH)],
                            start=(ch == 0), stop=(ch == CT - 1))
                    nc.vector.tensor_scalar_add(out=o_sb[:, tq, ds(nch * NCH, NCH)],
                                                in0=ps, scalar1=b_sb[:, tq:tq + 1])

        for mt in range(MT):
            ps = psS.tile([P, C], F32, tag="s", name="psv")
            for ch in range(CT):
                nc.tensor.matmul(ps,
                                 lhsT=xn_sb[:, ch, ts(mt, P)],
                                 rhs=wv_sb[:, ch, :],
                                 start=(ch == 0), stop=(ch == CT - 1))
            nc.vector.tensor_add(out=vT_sb[:, mt, :], in0=ps, in1=bv_sb)

        # ---------------- attention + output projection ----------------
        PAIR = 1  # score tiles per exp instruction
        for nch in range(NNCH):
            nsl = ds(nch * NCH, NCH)
            attn = psA.tile([P, CT, NCH], F32, tag="attn", name="attn")
            den = psD.tile([1, NCH], F32, tag="den", name="den")
            for mt0 in range(0, MT, PAIR):
                s = psS.tile([P, PAIR, NCH], F32, tag="s", name="s")
                for j in range(PAIR):
                    for ch in range(CT):
                        nc.tensor.matmul(s[:, j, :],
                                         lhsT=k_sb[:, ch, ts(mt0 + j, P)],
                                         rhs=q_sb[:, ch, nsl],
                                         start=(ch == 0), stop=(ch == CT - 1))
                e = work.tile([P, PAIR, NCH], F32R, tag="e", name="e")
                nc.scalar.activation(out=e, in_=s, func=AF.Exp, bias=zero_sb,
                                     scale=SCALE)
                for j in range(PAIR):
                    mt = mt0 + j
                    for ch in range(CT):
                        nc.tensor.matmul(attn[:, ch, :],
                                         lhsT=vT_sb[:, mt, ts(ch, P)],
                                         rhs=e[:, j, :],
                                         start=(mt == 0), stop=(mt == MT - 1))
                    nc.tensor.matmul(den, lhsT=ones_sb, rhs=e[:, j, :],
                                     start=(mt == 0), stop=(mt == MT - 1))

            den_sb = small.tile([1, NCH], F32, tag="den_sb", name="den_sb", bufs=2)
            nc.vector.tensor_copy(out=den_sb, in_=den)
            rden = small.tile([1, NCH], F32, tag="rden", name="rden", bufs=2)
            nc.vector.reciprocal(out=rden, in_=den_sb)
            rdenb = outp.tile([P, NCH], F32, tag="rdenb", name="rdenb", bufs=2)
            nc.gpsimd.partition_broadcast(rdenb, rden)

            atts = []
            for ch in range(CT):
                att = outp.tile([P, NCH], F32R, tag="att", name=f"att{ch}", bufs=3)
                nc.vector.tensor_copy(out=att, in_=attn[:, ch, :])
                atts.append(att)

            for co in range(CT):
                pj = psP.tile([P, NCH], F32, tag="proj", name="pj")
                for ch in range(CT):
                    nc.tensor.matmul(pj,
                                     lhsT=wo_sb[:, ch, ts(co, P)],
                                     rhs=atts[ch],
                                     start=(ch == 0), stop=(ch == CT - 1))
                f = outp.tile([P, NCH], F32, tag="fout", name="f", bufs=2)
                nc.vector.tensor_tensor(out=f, in0=pj, in1=rdenb, op=OP.mult)
                nc.vector.scalar_tensor_tensor(out=f, in0=f, scalar=bo_sb[:, co:co + 1],
                                               in1=x_sb[:, co, nsl],
                                               op0=OP.add, op1=OP.add)
                nc.sync.dma_start(out=out_d[ts(co, P), nsl], in_=f)


def build_program():
    nc = bacc.Bacc("TRN2", target_bir_lowering=False, debug=False, num_devices=B)
    d = {}

    def din(name, shape, dt_=F32):
        d[name] = nc.dram_tensor(name, list(shape), dt_, kind="ExternalInput").ap()

    din("x", (C, N))
    din("wq_t", (C, C), F32R)
    din("wk_t", (C, C), F32R)
    din("wv_t", (C, C), F32R)
    din("wo_t", (C, C), F32R)
    din("b_q", (CT, P, 1))
    din("b_k", (CT, P, 1))
    din("b_v", (C,))
    din("b_o", (CT, P, 1))
    din("gn_w", (CT, P, 1))
    din("gn_b", (CT, P, 1))
    din("fmask", (CT, P, G))
    din("bmask", (CT, G, P))
    din("ones_col", (P, 1), F32R)
    out_d = nc.dram_tensor("out", [C, N], F32, kind="ExternalOutput").ap()

    with tile.TileContext(nc) as tc:
        _emit(tc, d, out_d)
    nc.compile()
    return nc


_PROG = None


def _get_program():
    global _PROG
    if _PROG is None:
        _PROG = build_program()
    return _PROG


def make_in_maps(inputs):
    x = np.ascontiguousarray(np.asarray(inputs["x"], dtype=np.float32))
    w_qkv = np.asarray(inputs["w_qkv"], dtype=np.float32)
    b_qkv = np.asarray(inputs["b_qkv"], dtype=np.float32)
    w_out = np.asarray(inputs["w_out"], dtype=np.float32)
    b_out = np.asarray(inputs["b_out"], dtype=np.float32)
    gn_scale = np.asarray(inputs["gn_scale"], dtype=np.float32)
    gn_bias = np.asarray(inputs["gn_bias"], dtype=np.float32)

    fmask = np.zeros((CT, P, G), dtype=np.float32)
    for t in range(CT):
        for p in range(P):
            fmask[t, p, (t * P + p) // GSZ] = 1.0
    bmask = np.ascontiguousarray(fmask.transpose(0, 2, 1))

    common = {
        "wq_t": np.ascontiguousarray(w_qkv[0:C].T),
        "wk_t": np.ascontiguousarray(w_qkv[C:2 * C].T),
        "wv_t": np.ascontiguousarray(w_qkv[2 * C:3 * C].T),
        "wo_t": np.ascontiguousarray(w_out.T),
        "b_q": np.ascontiguousarray(b_qkv[0:C].reshape(CT, P, 1)),
        "b_k": np.ascontiguousarray(b_qkv[C:2 * C].reshape(CT, P, 1)),
        "b_v": np.ascontiguousarray(b_qkv[2 * C:3 * C]),
        "b_o": np.ascontiguousarray(b_out.reshape(CT, P, 1)),
        "gn_w": np.ascontiguousarray(gn_scale.reshape(CT, P, 1)),
        "gn_b": np.ascontiguousarray(gn_bias.reshape(CT, P, 1)),
        "fmask": fmask,
        "bmask": bmask,
        "ones_col": np.ones((P, 1), dtype=np.float32),
    }
    return [dict(common, x=np.ascontiguousarray(x[b].reshape(C, N)))
            for b in range(B)]


def run(inputs, trace=False):
    nc = _get_program()
    in_maps = make_in_maps(inputs)
    res = bass_utils.run_bass_kernel_spmd(nc, in_maps, core_ids=list(range(B)),
                                          trace=trace)
    out = np.stack([res.results[b]["out"] for b in range(B)])
    return out.reshape(B, C, HH, WW), res


def kernel(**inputs):
    out, _ = run(inputs, trace=False)
    return out
